# revision 33
# baseline (speedup 1.0000x reference)
"""GPT2 eager causal attention (B=2, S=2048, D=1024, H=16, HD=64) on 8 TRN2 NeuronCores.

Sharding (data + head/tensor parallel, per the problem's hint):
  core c -> (batch b = c//4, head-group g = c%4) -- 4 heads per group.

Per-core pipeline, software-pipelined per 512-token chunk so the in-order PE
stream never waits on a collective:
  chunk ch: QT/KT columns for the chunk (wq/wk^T @ xT), V row-tiles,
            scores^T = KT^T-slices @ QT-slices (exact causal: diagonal
            matmuls only cover the valid column suffix), exp on ScalarE,
            OT += V^T @ ST_exp with a ones-column producing the softmax
            denominator; normalize via reciprocal + rank-1 PE broadcast.
  Each head-pair's OT slice [128ch x 512tok] (bf16) is AllGathered across the
  4 same-batch cores as soon as both heads finish (8 small collectives/core);
  each core then computes the FULL-contraction c_proj for its own 128 tokens
  of each chunk (no cross-core reduction, no trailing ReduceScatter). The
  readback row offset is rank-dependent, selected at runtime via a dynamic
  DMA offset from partition_id. c_proj for chunk ch is emitted after
  attention ch+1 so the AllGather latency hides under compute.

x arrives pre-transposed from the host ([D, S]), so no device transposes are
needed; xT chunk loads interleave with weight loads across the two HWDGE
queues to keep the DMAHW lane round-robin from stalling. All matmuls bf16
with fp32 PSUM accumulation.
"""
from contextlib import ExitStack

import ml_dtypes
import numpy as np

import concourse.bacc as bacc
import concourse.mybir as mybir
import concourse.tile as tile
from concourse.bass import ds
from concourse.bass_utils import run_bass_kernel_spmd

F32 = mybir.dt.float32
BF16 = mybir.dt.bfloat16

B, S, D, H, HD = 2, 2048, 1024, 16, 64
N_CORES = 8
HG = 4               # heads per group
DG = HG * HD         # 256 q/k channels per group
VW = HG * (HD + 1)   # 260: 64 v-cols + 1 ones-col per head
NK = D // 128        # 8 contraction tiles over d
NS = S // 128        # 16 token tiles
CH = 512             # q-chunk (one PSUM bank of fp32)
NCH = S // CH        # 4
NRT = DG // 128      # 2 channel row-tiles per group


def _build(has_bv: bool, has_bp: bool, has_bqk: bool = False):
    nc = bacc.Bacc("TRN2", target_bir_lowering=False, debug=False, num_devices=N_CORES)

    x_d = nc.dram_tensor("x", [D, S], BF16, kind="ExternalInput").ap()  # pre-transposed host-side
    wq_d = nc.dram_tensor("wq", [D, DG], BF16, kind="ExternalInput").ap()
    wk_d = nc.dram_tensor("wk", [D, DG], BF16, kind="ExternalInput").ap()
    wv_d = nc.dram_tensor("wv", [D, VW], BF16, kind="ExternalInput").ap()
    wp_d = nc.dram_tensor("wp", [D, D], BF16, kind="ExternalInput").ap()
    bq_d = nc.dram_tensor("bq", [DG, 1], F32, kind="ExternalInput").ap()
    bk_d = nc.dram_tensor("bk", [DG, 1], F32, kind="ExternalInput").ap()
    bv_d = nc.dram_tensor("bv", [DG, 1], F32, kind="ExternalInput").ap()
    bp_d = nc.dram_tensor("bp", [128, D], F32, kind="ExternalInput").ap()
    mk_d = nc.dram_tensor("masks", [128, 128], BF16, kind="ExternalInput").ap()
    out_d = nc.dram_tensor("out", [NCH * 128, D], F32, kind="ExternalOutput").ap()

    EXP = mybir.ActivationFunctionType.Exp
    IDENT = mybir.ActivationFunctionType.Identity

    with ExitStack() as ctx:
        tc = ctx.enter_context(tile.TileContext(nc))
        wpool = ctx.enter_context(tc.tile_pool(name="w", bufs=1))
        big = ctx.enter_context(tc.tile_pool(name="big", bufs=8))
        qkvp = ctx.enter_context(tc.tile_pool(name="qkv", bufs=1))
        stp = ctx.enter_context(tc.tile_pool(name="stx", bufs=6))
        nrm = ctx.enter_context(tc.tile_pool(name="nrm", bufs=2))
        otfp = ctx.enter_context(tc.tile_pool(name="otf", bufs=4))
        outp = ctx.enter_context(tc.tile_pool(name="outp", bufs=3))
        ps_mm = ctx.enter_context(tc.tile_pool(name="psmm", bufs=3, space="PSUM"))
        ps_st = ctx.enter_context(tc.tile_pool(name="psst", bufs=3, space="PSUM"))
        ps_ot = ctx.enter_context(tc.tile_pool(name="psot", bufs=2, space="PSUM"))
        dram = ctx.enter_context(tc.tile_pool(name="dram", bufs=1, space="DRAM"))

        # ---- constants / weights -> SBUF, x (pre-transposed) -> SBUF
        # Emission interleaves the sync-queue xT loads with the act-queue
        # weight loads so the global round-robin DMAHW lane waits always
        # reference recently-completed DMAs (never a queue 30 positions back).
        wq_sb = wpool.tile([128, NK * DG], BF16)
        wk_sb = wpool.tile([128, NK * DG], BF16)
        wv_sb = wpool.tile([128, NK * VW], BF16)
        wp_sb = wpool.tile([128, NK * D], BF16)
        mk_sb = wpool.tile([128, 128], BF16)
        on_sb = wpool.tile([1, 64], F32)
        nc.vector.memset(on_sb[:], 1.0)
        bq_sb = wpool.tile([128, NRT], F32)
        bk_sb = wpool.tile([128, NRT], F32)
        bv_sb = wpool.tile([128, NRT], F32) if has_bv else None
        bp_sb = wpool.tile([128, D], F32) if has_bp else None

        xT = []
        for dt in range(NK):
            t = big.tile([128, S], BF16, tag="bigslot", name=f"xT{dt}")
            xT.append(t)

        def load_xt_chunk(sq):
            for dt in range(NK):
                nc.sync.dma_start(
                    xT[dt][:, sq * CH:(sq + 1) * CH],
                    x_d[dt * 128:(dt + 1) * 128, sq * CH:(sq + 1) * CH],
                )

        # interleave per-kt so the first QKV matmul (kt=0) unblocks earliest
        for kt in range(NK):
            nc.sync.dma_start(
                xT[kt][:, 0:CH],
                x_d[kt * 128:(kt + 1) * 128, 0:CH],
            )
            weng = nc.sync if kt == 0 else nc.scalar
            weng.dma_start(wq_sb[:, kt * DG:(kt + 1) * DG], wq_d[kt * 128:(kt + 1) * 128, :])
            weng.dma_start(wk_sb[:, kt * DG:(kt + 1) * DG], wk_d[kt * 128:(kt + 1) * 128, :])
        load_xt_chunk(1)
        for kt in range(NK):
            nc.scalar.dma_start(wv_sb[:, kt * VW:(kt + 1) * VW], wv_d[kt * 128:(kt + 1) * 128, :])
        nc.sync.dma_start(mk_sb[:], mk_d[:])
        for rt in range(NRT):
            nc.sync.dma_start(bq_sb[:, rt:rt + 1], bq_d[rt * 128:(rt + 1) * 128, :])
            nc.sync.dma_start(bk_sb[:, rt:rt + 1], bk_d[rt * 128:(rt + 1) * 128, :])
            if has_bv:
                nc.sync.dma_start(bv_sb[:, rt:rt + 1], bv_d[rt * 128:(rt + 1) * 128, :])
        load_xt_chunk(2)
        load_xt_chunk(3)
        for kt in range(NK):
            nc.scalar.dma_start(wp_sb[:, kt * D:(kt + 1) * D], wp_d[kt * 128:(kt + 1) * 128, :])
        if has_bp:
            nc.scalar.dma_start(bp_sb[:], bp_d[:])

        QT, KT = [], []
        for store, nm in ((QT, "q"), (KT, "k")):
            for rt in range(NRT):
                dst = qkvp.tile([128, S], BF16, tag=f"{nm}t{rt}", name=f"{nm}T{rt}")
                store.append(dst)
        V = [None] * NS
        OT = []
        for i in range(NRT):
            # own tag: must NOT alias the xT buffers -- OT writes begin while
            # later chunks' QKV still reads xT
            t = big.tile([128, S], BF16, tag="otslot", bufs=2, name=f"OT{i}")
            OT.append(t)

        # per (chunk, rt-half) AllGather buffers: issuing the gather for a
        # head-pair as soon as its two heads finish halves the latency exposed
        # after the last chunk's attention
        ag_ins, ag_outs = {}, {}
        for ch in range(NCH):
            for rt in range(NRT):
                gi = dram.tile([4 * 128, 128], BF16, tag=f"agin{ch}_{rt}", name=f"ag_in{ch}_{rt}")
                go = dram.tile([HG * 4 * 128, 128], BF16, tag=f"agout{ch}_{rt}", name=f"ag_out{ch}_{rt}")
                ag_ins[ch, rt] = gi
                ag_outs[ch, rt] = go

        def qkv_chunk(ch):
            # QT/KT columns for this chunk
            for store, w_sb, b_sb, nm in ((QT, wq_sb, bq_sb, "q"), (KT, wk_sb, bk_sb, "k")):
                for rt in range(NRT):
                    dst = store[rt]
                    ps = ps_mm.tile([128, CH], F32, tag="ps", name=f"ps{nm}{rt}_{ch}")
                    for kt in range(NK):
                        nc.tensor.matmul(
                            ps[:],
                            (w_sb[:, kt * DG + rt * 128: kt * DG + (rt + 1) * 128]),
                            (xT[kt][:, ch * CH:(ch + 1) * CH]),
                            start=(kt == 0), stop=(kt == NK - 1),
                        )
                    if has_bqk:
                        nc.scalar.activation(
                            dst[:, ch * CH:(ch + 1) * CH], ps[:], IDENT,
                            bias=b_sb[:, rt:rt + 1],
                        )
                    else:
                        nc.vector.tensor_copy(dst[:, ch * CH:(ch + 1) * CH], ps[:])
            # V row-tiles for this chunk's tokens
            for st in range(4 * ch, 4 * ch + 4):
                vt = qkvp.tile([128, VW], BF16, tag=f"v{st}", name=f"v{st}")
                ps = ps_mm.tile([128, CH], F32, tag="ps", name=f"psv{st}")
                for kt in range(NK):
                    nc.tensor.matmul(
                        ps[:, :VW],
                        (xT[kt][:, st * 128:(st + 1) * 128]),
                        (wv_sb[:, kt * VW:(kt + 1) * VW]),
                        start=(kt == 0), stop=(kt == NK - 1),
                    )
                nc.vector.tensor_copy(vt[:], ps[:, :VW])
                for hl in range(HG):
                    ones_col = vt[:, hl * (HD + 1) + HD: (hl + 1) * (HD + 1)].bitcast(mybir.dt.uint16)
                    nc.vector.memset(ones_col, 0x3F80)  # bits of bf16 1.0
                V[st] = vt

        def emit_ag_half(ch, rt):
            # ag_in rows 128*j + p <-> (token block j, channel rt*128+p)
            for j in range(4):
                nc.sync.dma_start(
                    ag_ins[ch, rt][128 * j: 128 * (j + 1), :],
                    OT[rt][:, ch * CH + j * 128: ch * CH + (j + 1) * 128],
                )
            nc.gpsimd.collective_compute(
                "AllGather",
                mybir.AluOpType.bypass,
                replica_groups=[[0, 1, 2, 3], [4, 5, 6, 7]],
                ins=[ag_ins[ch, rt].opt()],
                outs=[ag_outs[ch, rt].opt()],
            )

        def attention_chunk(ch):
            nkt = 4 * (ch + 1)
            pending = []

            def flush_pending():
                while pending:
                    pending.pop(0)()

            for hl in range(HG):
                qt = QT[hl // 2]
                ktile = KT[hl // 2]
                off = 64 * (hl % 2)
                ot_ps = ps_ot.tile([65, CH], F32, tag="ot", name=f"ot{ch}_{hl}")
                for kt in range(nkt):
                    st_ps = ps_st.tile([128, CH], F32, tag="st", name=f"st{ch}_{hl}_{kt}")
                    st_sb = stp.tile([128, CH], BF16, tag="stsb", name=f"se{ch}_{hl}_{kt}")
                    d = kt - 4 * ch
                    if d < 0:
                        nc.tensor.matmul(
                            st_ps[:],
                            (ktile[off:off + 64, kt * 128:(kt + 1) * 128]),
                            (qt[off:off + 64, ch * CH:(ch + 1) * CH]),
                            start=True, stop=True,
                        )
                        nc.scalar.activation(st_sb[:], st_ps[:], EXP, scale=0.125)
                    else:
                        # diagonal strip: only the valid column suffix is ever
                        # nonzero under causality -- matmul/exp just that part,
                        # zero the prefix, triangular-mask the diagonal block
                        nc.tensor.matmul(
                            st_ps[:, d * 128:],
                            (ktile[off:off + 64, kt * 128:(kt + 1) * 128]),
                            (qt[off:off + 64, ch * CH + d * 128:(ch + 1) * CH]),
                            start=True, stop=True,
                        )
                        if d > 0:
                            zc = st_sb[:, 0:d * 128].bitcast(mybir.dt.uint16)
                            nc.vector.memset(zc, 0)
                        nc.scalar.activation(st_sb[:, d * 128:], st_ps[:, d * 128:], EXP, scale=0.125)
                        nc.vector.tensor_mul(
                            st_sb[:, d * 128:(d + 1) * 128],
                            st_sb[:, d * 128:(d + 1) * 128],
                            mk_sb[:, 0:128],
                        )
                    nc.tensor.matmul(
                        ot_ps[:],
                        (V[kt][:, hl * (HD + 1):(hl + 1) * (HD + 1)]),
                        (st_sb[:]),
                        start=(kt == 0), stop=(kt == nkt - 1),
                    )
                    if kt == 1:
                        # previous head's deferred normalize lands here so the
                        # PE never stalls on the DVE den->reciprocal latency
                        flush_pending()
                den = nrm.tile([1, CH], F32, tag="den", name=f"den{ch}_{hl}")
                nc.vector.tensor_copy(den[:], ot_ps[64:65, :])
                rden = nrm.tile([1, CH], F32, tag="rden", name=f"rden{ch}_{hl}")
                nc.vector.reciprocal_approx_fast(rden[:], den[:])
                ot_sb = nrm.tile([64, CH], BF16, tag="otsb", name=f"otsb{ch}_{hl}")
                nc.vector.tensor_copy(ot_sb[:], ot_ps[0:64, :])

                def finish(hl=hl, off=off, rden=rden, ot_sb=ot_sb):
                    rbc = ps_st.tile([64, CH], F32, tag="st", name=f"rbc{ch}_{hl}")
                    nc.tensor.matmul(rbc[:], on_sb[:], rden[:], start=True, stop=True)
                    dst = OT[hl // 2][off:off + 64, ch * CH:(ch + 1) * CH]
                    nc.vector.tensor_mul(dst, ot_sb[:], rbc[:])
                    if has_bv:
                        nc.vector.tensor_scalar_add(dst, dst, bv_sb[off:off + 64, hl // 2: hl // 2 + 1])
                    if hl % 2 == 1:
                        emit_ag_half(ch, hl // 2)

                pending.append(finish)
            flush_pending()

        otf_tiles = {}
        # group index of this core (SPMD: runtime value) -- selects which
        # 128-token block of each rank's gathered OT slice this core projects
        g_idx = nc.scalar.partition_id() % 4

        def emit_readback(ch):
            # rank r's contribution for our token block sits at rows
            # 512*r + 128*g of the rt-half ag_out. One tile per channel block:
            # a shared tile would make every c_proj matmul wait on ALL eight
            # readback DMAs (i.e. on the last AllGather half).
            tiles = [None] * 8
            for rt in range(NRT):
                for r in range(HG):
                    c2 = 2 * r + rt
                    otf = otfp.tile([128, 128], BF16, tag=f"otf{c2}", name=f"otf{ch}_{c2}")
                    tiles[c2] = otf
                    nc.scalar.dma_start(
                        otf[:],
                        ag_outs[ch, rt][ds(g_idx * 128 + 512 * r, 128), :],
                    )
            otf_tiles[ch] = tiles

        def cproj_chunk(ch):
            tiles = otf_tiles[ch]
            order = [2 * r for r in range(HG)] + [2 * r + 1 for r in range(HG)]
            for n in range(2):
                po = ps_mm.tile([128, CH], F32, tag="ps", name=f"po{ch}_{n}")
                for i, c2 in enumerate(order):
                    # rt-0 channels first: they arrive with the earlier AG
                    # half, so the chain starts while the second AG flies
                    nc.tensor.matmul(
                        po[:],
                        (tiles[c2][:]),
                        (wp_sb[:, c2 * D + n * CH: c2 * D + (n + 1) * CH]),
                        start=(i == 0), stop=(i == 7),
                    )
                ob = outp.tile([128, CH], F32, tag="ob", name=f"ob{ch}_{n}")
                if has_bp:
                    nc.vector.tensor_add(ob[:], po[:], bp_sb[:, n * CH:(n + 1) * CH])
                else:
                    nc.vector.tensor_copy(ob[:], po[:])
                nc.sync.dma_start(out_d[ch * 128:(ch + 1) * 128, n * CH:(n + 1) * CH], ob[:])

        for ch in range(NCH):
            qkv_chunk(ch)
            attention_chunk(ch)
        # ALL c_proj work happens at the tail: the middle of the kernel is
        # PE-bound, so any c_proj placed there lengthens it 1:1, while here
        # chunks 0-2 (whose gathers completed long ago) become pure fill for
        # the final AllGather's peer-skew latency; chunk 3's rt1-half matmuls
        # (the only AG-dependent ones) then start almost immediately.
        for ch in range(NCH - 1):
            emit_readback(ch)
            cproj_chunk(ch)
        emit_readback(NCH - 1)
        cproj_chunk(NCH - 1)

    nc.compile()
    return nc


_prog_cache = {}


def _get_prog(has_bv, has_bp, has_bqk):
    key = (has_bv, has_bp, has_bqk)
    if key not in _prog_cache:
        _prog_cache[key] = _build(has_bv, has_bp, has_bqk)
    return _prog_cache[key]


def _prepare(x, w_attn, b_attn, w_proj, b_proj):
    x = np.asarray(x, dtype=np.float32)
    w_attn = np.asarray(w_attn, dtype=np.float32)
    b_attn = np.asarray(b_attn, dtype=np.float32)
    w_proj = np.asarray(w_proj, dtype=np.float32)
    b_proj = np.asarray(b_proj, dtype=np.float32)

    has_bv = bool(np.any(b_attn[2 * D:]))
    has_bp = bool(np.any(b_proj))
    has_bqk = bool(np.any(b_attn[:2 * D]))
    nc = _get_prog(has_bv, has_bp, has_bqk)

    ii = np.arange(128)[:, None]
    jj = np.arange(128)[None, :]
    masks = (jj >= ii).astype(np.float32).astype(ml_dtypes.bfloat16)
    bp_tile = np.broadcast_to(b_proj, (128, D)).astype(np.float32)
    wp_full = np.ascontiguousarray(w_proj).astype(ml_dtypes.bfloat16)

    in_maps = []
    for c in range(N_CORES):
        b, g = divmod(c, 4)
        q0 = g * DG
        k0 = D + g * DG
        v0 = 2 * D + g * DG
        wv_ext = np.zeros((D, VW), dtype=np.float32)
        for hl in range(HG):
            wv_ext[:, hl * (HD + 1):hl * (HD + 1) + HD] = w_attn[:, v0 + hl * HD: v0 + (hl + 1) * HD]
        in_maps.append({
            "x": np.ascontiguousarray(x[b].T).astype(ml_dtypes.bfloat16),
            "wq": np.ascontiguousarray(w_attn[:, q0:q0 + DG]).astype(ml_dtypes.bfloat16),
            "wk": np.ascontiguousarray(w_attn[:, k0:k0 + DG]).astype(ml_dtypes.bfloat16),
            "wv": wv_ext.astype(ml_dtypes.bfloat16),
            "wp": wp_full,
            "bq": np.ascontiguousarray(b_attn[q0:q0 + DG, None]),
            "bk": np.ascontiguousarray(b_attn[k0:k0 + DG, None]),
            "bv": np.ascontiguousarray(b_attn[v0:v0 + DG, None]),
            "bp": bp_tile,
            "masks": masks,
        })
    return nc, in_maps


def _assemble(results):
    out = np.empty((B, S, D), dtype=np.float32)
    for c in range(N_CORES):
        b, g = divmod(c, 4)
        o = results[c]["out"]
        for ch in range(NCH):
            tok = ch * CH + g * 128
            out[b, tok:tok + 128, :] = o[ch * 128:(ch + 1) * 128, :]
    return out


def kernel(x, w_attn, b_attn, w_proj, b_proj):
    nc, in_maps = _prepare(x, w_attn, b_attn, w_proj, b_proj)
    res = run_bass_kernel_spmd(nc, in_maps, list(range(N_CORES)))
    return _assemble(res.results)


# revision 34
# speedup vs baseline: 1.0783x; 1.0783x over previous
"""GPT2 eager causal attention (B=2, S=2048, D=1024, H=16, HD=64) on 8 TRN2 NeuronCores.

Sharding (data + head/tensor parallel, per the problem's hint):
  core c -> (batch b = c//4, head-group g = c%4) -- 4 heads per group.

Per-core pipeline, software-pipelined per 512-token chunk so the in-order PE
stream never waits on a collective:
  chunk ch: QT/KT columns for the chunk (wq/wk^T @ xT), V row-tiles,
            scores^T = KT^T-slices @ QT-slices (exact causal: diagonal
            matmuls only cover the valid column suffix), exp on ScalarE,
            OT += V^T @ ST_exp with a ones-column producing the softmax
            denominator; normalize via reciprocal + rank-1 PE broadcast.
  Each head-pair's OT slice [128ch x 512tok] (bf16) is AllGathered across the
  4 same-batch cores as soon as both heads finish (8 small collectives/core);
  each core then computes the FULL-contraction c_proj for its own 128 tokens
  of each chunk (no cross-core reduction, no trailing ReduceScatter). The
  readback row offset is rank-dependent, selected at runtime via a dynamic
  DMA offset from partition_id. c_proj for chunk ch is emitted after
  attention ch+1 so the AllGather latency hides under compute.

x arrives pre-transposed from the host ([D, S]), so no device transposes are
needed; xT chunk loads interleave with weight loads across the two HWDGE
queues to keep the DMAHW lane round-robin from stalling. All matmuls bf16
with fp32 PSUM accumulation.
"""
from contextlib import ExitStack

import ml_dtypes
import numpy as np

import concourse.bacc as bacc
import concourse.mybir as mybir
import concourse.tile as tile
from concourse.bass import ds
from concourse.bass_utils import run_bass_kernel_spmd

F32 = mybir.dt.float32
BF16 = mybir.dt.bfloat16

B, S, D, H, HD = 2, 2048, 1024, 16, 64
N_CORES = 8
HG = 4               # heads per group
DG = HG * HD         # 256 q/k channels per group
VW = HG * (HD + 1)   # 260: 64 v-cols + 1 ones-col per head
NK = D // 128        # 8 contraction tiles over d
NS = S // 128        # 16 token tiles
CH = 512             # q-chunk (one PSUM bank of fp32)
NCH = S // CH        # 4
NRT = DG // 128      # 2 channel row-tiles per group


def _build(has_bv: bool, has_bp: bool, has_bqk: bool = False):
    nc = bacc.Bacc("TRN2", target_bir_lowering=False, debug=False, num_devices=N_CORES)

    x_d = nc.dram_tensor("x", [D, S], BF16, kind="ExternalInput").ap()  # pre-transposed host-side
    wq_d = nc.dram_tensor("wq", [D, DG], BF16, kind="ExternalInput").ap()
    wk_d = nc.dram_tensor("wk", [D, DG], BF16, kind="ExternalInput").ap()
    wv_d = nc.dram_tensor("wv", [D, VW], BF16, kind="ExternalInput").ap()
    wp_d = nc.dram_tensor("wp", [D, D], BF16, kind="ExternalInput").ap()
    bq_d = nc.dram_tensor("bq", [DG, 1], F32, kind="ExternalInput").ap()
    bk_d = nc.dram_tensor("bk", [DG, 1], F32, kind="ExternalInput").ap()
    bv_d = nc.dram_tensor("bv", [DG, 1], F32, kind="ExternalInput").ap()
    bp_d = nc.dram_tensor("bp", [128, D], F32, kind="ExternalInput").ap()
    mk_d = nc.dram_tensor("masks", [128, 128], BF16, kind="ExternalInput").ap()
    out_d = nc.dram_tensor("out", [NCH * 128, D], F32, kind="ExternalOutput").ap()

    EXP = mybir.ActivationFunctionType.Exp
    IDENT = mybir.ActivationFunctionType.Identity

    with ExitStack() as ctx:
        tc = ctx.enter_context(tile.TileContext(nc))
        wpool = ctx.enter_context(tc.tile_pool(name="w", bufs=1))
        big = ctx.enter_context(tc.tile_pool(name="big", bufs=8))
        qkvp = ctx.enter_context(tc.tile_pool(name="qkv", bufs=1))
        stp = ctx.enter_context(tc.tile_pool(name="stx", bufs=6))
        nrm = ctx.enter_context(tc.tile_pool(name="nrm", bufs=2))
        otfp = ctx.enter_context(tc.tile_pool(name="otf", bufs=4))
        outp = ctx.enter_context(tc.tile_pool(name="outp", bufs=3))
        ps_mm = ctx.enter_context(tc.tile_pool(name="psmm", bufs=3, space="PSUM"))
        ps_st = ctx.enter_context(tc.tile_pool(name="psst", bufs=3, space="PSUM"))
        ps_ot = ctx.enter_context(tc.tile_pool(name="psot", bufs=2, space="PSUM"))
        dram = ctx.enter_context(tc.tile_pool(name="dram", bufs=1, space="DRAM"))

        # ---- constants / weights -> SBUF, x (pre-transposed) -> SBUF
        # Emission interleaves the sync-queue xT loads with the act-queue
        # weight loads so the global round-robin DMAHW lane waits always
        # reference recently-completed DMAs (never a queue 30 positions back).
        wq_sb = wpool.tile([128, NK * DG], BF16)
        wk_sb = wpool.tile([128, NK * DG], BF16)
        wv_sb = wpool.tile([128, NK * VW], BF16)
        wp_sb = wpool.tile([128, NK * D], BF16)
        mk_sb = wpool.tile([128, 128], BF16)
        on_sb = wpool.tile([1, 64], F32)
        nc.vector.memset(on_sb[:], 1.0)
        bq_sb = wpool.tile([128, NRT], F32)
        bk_sb = wpool.tile([128, NRT], F32)
        bv_sb = wpool.tile([128, NRT], F32) if has_bv else None
        bp_sb = wpool.tile([128, D], F32) if has_bp else None

        xT = []
        for dt in range(NK):
            t = big.tile([128, S], BF16, tag="bigslot", name=f"xT{dt}")
            xT.append(t)

        def load_xt_chunk(sq):
            for dt in range(NK):
                nc.sync.dma_start(
                    xT[dt][:, sq * CH:(sq + 1) * CH],
                    x_d[dt * 128:(dt + 1) * 128, sq * CH:(sq + 1) * CH],
                )

        # interleave per-kt so the first QKV matmul (kt=0) unblocks earliest
        for kt in range(NK):
            nc.sync.dma_start(
                xT[kt][:, 0:CH],
                x_d[kt * 128:(kt + 1) * 128, 0:CH],
            )
            weng = nc.sync if kt == 0 else nc.scalar
            weng.dma_start(wq_sb[:, kt * DG:(kt + 1) * DG], wq_d[kt * 128:(kt + 1) * 128, :])
            weng.dma_start(wk_sb[:, kt * DG:(kt + 1) * DG], wk_d[kt * 128:(kt + 1) * 128, :])
        load_xt_chunk(1)
        for kt in range(NK):
            nc.scalar.dma_start(wv_sb[:, kt * VW:(kt + 1) * VW], wv_d[kt * 128:(kt + 1) * 128, :])
        nc.sync.dma_start(mk_sb[:], mk_d[:])
        for rt in range(NRT):
            nc.sync.dma_start(bq_sb[:, rt:rt + 1], bq_d[rt * 128:(rt + 1) * 128, :])
            nc.sync.dma_start(bk_sb[:, rt:rt + 1], bk_d[rt * 128:(rt + 1) * 128, :])
            if has_bv:
                nc.sync.dma_start(bv_sb[:, rt:rt + 1], bv_d[rt * 128:(rt + 1) * 128, :])
        load_xt_chunk(2)
        load_xt_chunk(3)
        for kt in range(NK):
            nc.scalar.dma_start(wp_sb[:, kt * D:(kt + 1) * D], wp_d[kt * 128:(kt + 1) * 128, :])
        if has_bp:
            nc.scalar.dma_start(bp_sb[:], bp_d[:])

        QT, KT = [], []
        for store, nm in ((QT, "q"), (KT, "k")):
            for rt in range(NRT):
                dst = qkvp.tile([128, S], BF16, tag=f"{nm}t{rt}", name=f"{nm}T{rt}")
                store.append(dst)
        V = [None] * NS
        OT = []
        for i in range(NRT):
            # own tag: must NOT alias the xT buffers -- OT writes begin while
            # later chunks' QKV still reads xT
            t = big.tile([128, S], BF16, tag="otslot", bufs=2, name=f"OT{i}")
            OT.append(t)

        # per (chunk, rt-half) AllGather buffers: issuing the gather for a
        # head-pair as soon as its two heads finish halves the latency exposed
        # after the last chunk's attention
        ag_ins, ag_outs = {}, {}
        for ch in range(NCH):
            for rt in range(NRT):
                gi = dram.tile([4 * 128, 128], BF16, tag=f"agin{ch}_{rt}", name=f"ag_in{ch}_{rt}")
                go = dram.tile([HG * 4 * 128, 128], BF16, tag=f"agout{ch}_{rt}", name=f"ag_out{ch}_{rt}")
                ag_ins[ch, rt] = gi
                ag_outs[ch, rt] = go

        def qkv_chunk(ch):
            # QT/KT columns for this chunk
            for store, w_sb, b_sb, nm in ((QT, wq_sb, bq_sb, "q"), (KT, wk_sb, bk_sb, "k")):
                for rt in range(NRT):
                    dst = store[rt]
                    ps = ps_mm.tile([128, CH], F32, tag="ps", name=f"ps{nm}{rt}_{ch}")
                    for kt in range(NK):
                        nc.tensor.matmul(
                            ps[:],
                            (w_sb[:, kt * DG + rt * 128: kt * DG + (rt + 1) * 128]),
                            (xT[kt][:, ch * CH:(ch + 1) * CH]),
                            start=(kt == 0), stop=(kt == NK - 1),
                        )
                    if has_bqk:
                        nc.scalar.activation(
                            dst[:, ch * CH:(ch + 1) * CH], ps[:], IDENT,
                            bias=b_sb[:, rt:rt + 1],
                        )
                    else:
                        nc.vector.tensor_copy(dst[:, ch * CH:(ch + 1) * CH], ps[:])
            # V row-tiles for this chunk's tokens
            for st in range(4 * ch, 4 * ch + 4):
                vt = qkvp.tile([128, VW], BF16, tag=f"v{st}", name=f"v{st}")
                ps = ps_mm.tile([128, CH], F32, tag="ps", name=f"psv{st}")
                for kt in range(NK):
                    nc.tensor.matmul(
                        ps[:, :VW],
                        (xT[kt][:, st * 128:(st + 1) * 128]),
                        (wv_sb[:, kt * VW:(kt + 1) * VW]),
                        start=(kt == 0), stop=(kt == NK - 1),
                    )
                nc.vector.tensor_copy(vt[:], ps[:, :VW])
                for hl in range(HG):
                    ones_col = vt[:, hl * (HD + 1) + HD: (hl + 1) * (HD + 1)].bitcast(mybir.dt.uint16)
                    nc.vector.memset(ones_col, 0x3F80)  # bits of bf16 1.0
                V[st] = vt

        def emit_ag_half(ch, rt):
            # ag_in rows 128*j + p <-> (token block j, channel rt*128+p)
            for j in range(4):
                nc.sync.dma_start(
                    ag_ins[ch, rt][128 * j: 128 * (j + 1), :],
                    OT[rt][:, ch * CH + j * 128: ch * CH + (j + 1) * 128],
                )
            nc.gpsimd.collective_compute(
                "AllGather",
                mybir.AluOpType.bypass,
                replica_groups=[[0, 1, 2, 3], [4, 5, 6, 7]],
                ins=[ag_ins[ch, rt].opt()],
                outs=[ag_outs[ch, rt].opt()],
            )

        def attention_chunk(ch):
            nkt = 4 * (ch + 1)
            pending = []

            def flush_pending():
                while pending:
                    pending.pop(0)()

            for hl in range(HG):
                qt = QT[hl // 2]
                ktile = KT[hl // 2]
                off = 64 * (hl % 2)
                ot_ps = ps_ot.tile([65, CH], F32, tag="ot", name=f"ot{ch}_{hl}")
                for kt in range(nkt):
                    st_ps = ps_st.tile([128, CH], F32, tag="st", name=f"st{ch}_{hl}_{kt}")
                    st_sb = stp.tile([128, CH], BF16, tag="stsb", name=f"se{ch}_{hl}_{kt}")
                    d = kt - 4 * ch
                    if d < 0:
                        nc.tensor.matmul(
                            st_ps[:],
                            (ktile[off:off + 64, kt * 128:(kt + 1) * 128]),
                            (qt[off:off + 64, ch * CH:(ch + 1) * CH]),
                            start=True, stop=True,
                        )
                        nc.scalar.activation(st_sb[:], st_ps[:], EXP, scale=0.125)
                    else:
                        # diagonal strip: only the valid column suffix is ever
                        # nonzero under causality -- matmul/exp just that part,
                        # zero the prefix, triangular-mask the diagonal block
                        nc.tensor.matmul(
                            st_ps[:, d * 128:],
                            (ktile[off:off + 64, kt * 128:(kt + 1) * 128]),
                            (qt[off:off + 64, ch * CH + d * 128:(ch + 1) * CH]),
                            start=True, stop=True,
                        )
                        if d > 0:
                            zc = st_sb[:, 0:d * 128].bitcast(mybir.dt.uint16)
                            nc.vector.memset(zc, 0)
                        nc.scalar.activation(st_sb[:, d * 128:], st_ps[:, d * 128:], EXP, scale=0.125)
                        nc.vector.tensor_mul(
                            st_sb[:, d * 128:(d + 1) * 128],
                            st_sb[:, d * 128:(d + 1) * 128],
                            mk_sb[:, 0:128],
                        )
                    nc.tensor.matmul(
                        ot_ps[:],
                        (V[kt][:, hl * (HD + 1):(hl + 1) * (HD + 1)]),
                        (st_sb[:]),
                        start=(kt == 0), stop=(kt == nkt - 1),
                    )
                    if kt == 1:
                        # previous head's deferred normalize lands here so the
                        # PE never stalls on the DVE den->reciprocal latency
                        flush_pending()
                den = nrm.tile([1, CH], F32, tag="den", name=f"den{ch}_{hl}")
                nc.vector.tensor_copy(den[:], ot_ps[64:65, :])
                rden = nrm.tile([1, CH], F32, tag="rden", name=f"rden{ch}_{hl}")
                nc.vector.reciprocal_approx_fast(rden[:], den[:])
                ot_sb = nrm.tile([64, CH], BF16, tag="otsb", name=f"otsb{ch}_{hl}")
                nc.vector.tensor_copy(ot_sb[:], ot_ps[0:64, :])

                def finish(hl=hl, off=off, rden=rden, ot_sb=ot_sb):
                    rbc = ps_st.tile([64, CH], F32, tag="st", name=f"rbc{ch}_{hl}")
                    nc.tensor.matmul(rbc[:], on_sb[:], rden[:], start=True, stop=True)
                    dst = OT[hl // 2][off:off + 64, ch * CH:(ch + 1) * CH]
                    nc.vector.tensor_mul(dst, ot_sb[:], rbc[:])
                    if has_bv:
                        nc.vector.tensor_scalar_add(dst, dst, bv_sb[off:off + 64, hl // 2: hl // 2 + 1])
                    if hl % 2 == 1:
                        emit_ag_half(ch, hl // 2)

                pending.append(finish)
            flush_pending()

        otf_tiles = {}
        # group index of this core (SPMD: runtime value) -- selects which
        # 128-token block of each rank's gathered OT slice this core projects
        g_idx = nc.scalar.partition_id() % 4

        def emit_readback(ch):
            # rank r's contribution for our token block sits at rows
            # 512*r + 128*g of the rt-half ag_out. One tile per channel block:
            # a shared tile would make every c_proj matmul wait on ALL eight
            # readback DMAs (i.e. on the last AllGather half).
            tiles = [None] * 8
            for rt in range(NRT):
                for r in range(HG):
                    c2 = 2 * r + rt
                    otf = otfp.tile([128, 128], BF16, tag=f"otf{c2}", name=f"otf{ch}_{c2}")
                    tiles[c2] = otf
                    nc.scalar.dma_start(
                        otf[:],
                        ag_outs[ch, rt][ds(g_idx * 128 + 512 * r, 128), :],
                    )
            otf_tiles[ch] = tiles

        def cproj_chunk(ch):
            tiles = otf_tiles[ch]
            order = [2 * r for r in range(HG)] + [2 * r + 1 for r in range(HG)]
            for n in range(2):
                po = ps_mm.tile([128, CH], F32, tag="ps", name=f"po{ch}_{n}")
                for i, c2 in enumerate(order):
                    # rt-0 channels first: they arrive with the earlier AG
                    # half, so the chain starts while the second AG flies
                    nc.tensor.matmul(
                        po[:],
                        (tiles[c2][:]),
                        (wp_sb[:, c2 * D + n * CH: c2 * D + (n + 1) * CH]),
                        start=(i == 0), stop=(i == 7),
                    )
                ob = outp.tile([128, CH], F32, tag="ob", name=f"ob{ch}_{n}")
                if has_bp:
                    nc.vector.tensor_add(ob[:], po[:], bp_sb[:, n * CH:(n + 1) * CH])
                else:
                    nc.vector.tensor_copy(ob[:], po[:])
                nc.sync.dma_start(out_d[ch * 128:(ch + 1) * 128, n * CH:(n + 1) * CH], ob[:])

        for ch in range(NCH):
            qkv_chunk(ch)
            if ch == 3:
                # chunk-0 c_proj lands here: its AllGathers finish late
                # (startup peer skew ~100us), so never put it before attention
                emit_readback(0)
                cproj_chunk(0)
            attention_chunk(ch)
            if ch >= 2:
                # chunk ch-1's gathers completed a full chunk ago; chunk 2's
                # c_proj doubles as fill for the final AllGather's latency
                emit_readback(ch - 1)
                cproj_chunk(ch - 1)
        emit_readback(NCH - 1)
        cproj_chunk(NCH - 1)

    nc.compile()
    return nc


_prog_cache = {}


def _get_prog(has_bv, has_bp, has_bqk):
    key = (has_bv, has_bp, has_bqk)
    if key not in _prog_cache:
        _prog_cache[key] = _build(has_bv, has_bp, has_bqk)
    return _prog_cache[key]


def _prepare(x, w_attn, b_attn, w_proj, b_proj):
    x = np.asarray(x, dtype=np.float32)
    w_attn = np.asarray(w_attn, dtype=np.float32)
    b_attn = np.asarray(b_attn, dtype=np.float32)
    w_proj = np.asarray(w_proj, dtype=np.float32)
    b_proj = np.asarray(b_proj, dtype=np.float32)

    has_bv = bool(np.any(b_attn[2 * D:]))
    has_bp = bool(np.any(b_proj))
    has_bqk = bool(np.any(b_attn[:2 * D]))
    nc = _get_prog(has_bv, has_bp, has_bqk)

    ii = np.arange(128)[:, None]
    jj = np.arange(128)[None, :]
    masks = (jj >= ii).astype(np.float32).astype(ml_dtypes.bfloat16)
    bp_tile = np.broadcast_to(b_proj, (128, D)).astype(np.float32)
    wp_full = np.ascontiguousarray(w_proj).astype(ml_dtypes.bfloat16)

    in_maps = []
    for c in range(N_CORES):
        b, g = divmod(c, 4)
        q0 = g * DG
        k0 = D + g * DG
        v0 = 2 * D + g * DG
        wv_ext = np.zeros((D, VW), dtype=np.float32)
        for hl in range(HG):
            wv_ext[:, hl * (HD + 1):hl * (HD + 1) + HD] = w_attn[:, v0 + hl * HD: v0 + (hl + 1) * HD]
        in_maps.append({
            "x": np.ascontiguousarray(x[b].T).astype(ml_dtypes.bfloat16),
            "wq": np.ascontiguousarray(w_attn[:, q0:q0 + DG]).astype(ml_dtypes.bfloat16),
            "wk": np.ascontiguousarray(w_attn[:, k0:k0 + DG]).astype(ml_dtypes.bfloat16),
            "wv": wv_ext.astype(ml_dtypes.bfloat16),
            "wp": wp_full,
            "bq": np.ascontiguousarray(b_attn[q0:q0 + DG, None]),
            "bk": np.ascontiguousarray(b_attn[k0:k0 + DG, None]),
            "bv": np.ascontiguousarray(b_attn[v0:v0 + DG, None]),
            "bp": bp_tile,
            "masks": masks,
        })
    return nc, in_maps


def _assemble(results):
    out = np.empty((B, S, D), dtype=np.float32)
    for c in range(N_CORES):
        b, g = divmod(c, 4)
        o = results[c]["out"]
        for ch in range(NCH):
            tok = ch * CH + g * 128
            out[b, tok:tok + 128, :] = o[ch * 128:(ch + 1) * 128, :]
    return out


def kernel(x, w_attn, b_attn, w_proj, b_proj):
    nc, in_maps = _prepare(x, w_attn, b_attn, w_proj, b_proj)
    res = run_bass_kernel_spmd(nc, in_maps, list(range(N_CORES)))
    return _assemble(res.results)
